# revision 18
# baseline (speedup 1.0000x reference)
"""Trainium2 Bass kernel for nn_ConvPlus1d (dense_cnn).

Algorithm (mathematically identical to the reference, derived analytically):

  The reference synthesizes per-sample conv weights:
      kern[b]   = mean_L(depthwise_conv(x))        -> [B, C_IN, K]
      w_in[b]   = W_in @ kern[b]                   -> [B, C_IN, K]
      w_out[b]  = <W_out, kern[b]>                 -> [B, C_OUT]
      bias[b]   = <W_bias, kern[b]>                -> [B, C_OUT]
      weight[b, o, c, k] = w_in[b, c, k] * w_out[b, o]     (rank-1!)
      y[b] = conv1d(x[b], weight[b], pad=1) + bias[b]

  Exact simplifications used here:
  1) mean over L of a pad-1 depthwise conv only needs per-channel sums and
     the first/last elements: kern (and therefore all synthesized params)
     are LINEAR in (S, E, F) with host-precomputable coefficient matrices.
     E/F are single input columns, shipped pre-gathered; S is reduced on
     device from the bf16 copy of x.
  2) The per-sample conv weight is rank-1 across (o) x (c,k).

  Device program per sample (data-parallel over batch, 4 samples/core):
      xs bf16 [64, L+2]  (stats), x8 fp8 [64, 2, L+2] (conv; plane 1 is
      plane 0 shifted left one column, so a DoubleRow matmul consumes two
      taps at 0.5 cycles/row)
      S: 4 chunk reduces (2 on DVE, 2 via ACT accum_out)
      params[1,320] = S^T M_S + [E|F]^T M_EF      (PE fp32r)
      bias[128,1]   = Mb_S^T S + Mb_EF^T [E|F]    (PE fp32, column out)
      W01dr fp8 [64, 2*128] = [w_in_k0 | w_in_k1] outer w_out, x 2^16
      W2Z   fp8 [64, 2*128] = [w_in_k2 | 0] outer w_out, x 2^16
      conv: per 512 tile, 2 fp8 DoubleRow matmuls (taps 0+1, tap 2+zero)
      PSUM is a 4-bank [128,2048] tile per 4-tile group; one eviction
      instruction per group applies x 2^-16 and the bias (ACT/DVE
      alternating), writing y in bf16.
  Host upcasts y bf16 -> fp32 (rel tol is 2e-2; measured pipeline error
  ~2e-3, dominated by bf16 quantization of x and the bf16 y store).

Sharding: batch 32 -> 8 cores x 4 samples, maker params replicated.
"""

import sys

import ml_dtypes
import numpy as np

sys.path.insert(0, "/opt/trn_rl_repo")

import concourse.bacc as bacc  # noqa: E402
import concourse.tile as tile  # noqa: E402
from concourse import mybir  # noqa: E402
from concourse.bass_utils import run_bass_kernel_spmd  # noqa: E402

B, C_IN, C_OUT, K, L = 32, 64, 128, 3, 8192
N_CORES = 8
BS = B // N_CORES          # samples per core
NT = 512                   # matmul moving-dim tile (one PSUM bank of fp32)
NTILES = L // NT
GROUP = 2                  # conv tiles per 2-bank PSUM tile
GW = NT * GROUP            # 2048 output columns per group
NCHUNK = 4                 # x load / reduce chunks
CW = 2048                  # chunk width (last chunk is CW+2)
WSCALE = 65536.0           # fp8 weight scale (Wtap rms ~3e-7 -> ~0.02)
L8 = L + 16                # fp8 plane length, 16B-aligned (pad cols are 0)

F32 = mybir.dt.float32
F32R = mybir.dt.float32r
BF16 = mybir.dt.bfloat16
FP8 = mybir.dt.float8e4
BF16_NP = ml_dtypes.bfloat16
FP8_NP = ml_dtypes.float8_e4m3
DR = mybir.MatmulPerfMode.DoubleRow


def _host_precompute(W_kernel, W_in, W_out, W_bias):
    """Fold the maker parameters into linear maps on the stats (S, E, F).

    params layout: [w_in k=0 (64) | k=1 (64) | k=2 (64) | w_out (128)].
    Returns M_S [64,320], M_EF [128,320] (rows 0:64 E, 64:128 F coeffs),
    Mb_S [64,128], Mb_EF [128,128].
    """
    Wk = W_kernel.reshape(C_IN, K, K).astype(np.float64)     # [c, j, t]
    P = (Wk[:, :, 0] + Wk[:, :, 1] + Wk[:, :, 2]) / L        # coeff on S
    Q = -Wk[:, :, 0] / L                                     # coeff on E
    R = -Wk[:, :, 2] / L                                     # coeff on F

    Win = W_in[:, :, 0].astype(np.float64)                   # [c, c']

    def m_in(Xc):   # -> [c', k*64+c]
        return np.einsum("cp,pk->pkc", Win, Xc).reshape(C_IN, K * C_IN)

    def m_out(Xc, W):  # -> [c', o]
        return np.einsum("ock,ck->co", W.astype(np.float64), Xc)

    def mm(Xc):
        return np.concatenate([m_in(Xc), m_out(Xc, W_out)], axis=1)  # [64,320]

    M_S = mm(P).astype(np.float32)
    M_EF = np.concatenate([mm(Q), mm(R)], axis=0).astype(np.float32)
    Mb_S = m_out(P, W_bias).astype(np.float32)
    Mb_EF = np.concatenate(
        [m_out(Q, W_bias), m_out(R, W_bias)], axis=0).astype(np.float32)
    return M_S, M_EF, Mb_S, Mb_EF


_CACHE = {}


def _build_module():
    if "nc" in _CACHE:
        return _CACHE["nc"]
    nc = bacc.Bacc("TRN2", target_bir_lowering=False, debug=False)

    # host supplies x pre-padded with one zero column on each side
    x_d = nc.dram_tensor("x", [BS, C_IN, L + 2], BF16,
                         kind="ExternalInput").ap()
    # fp8 conv input: plane 0 = xpad, plane 1 = xpad shifted left by 1
    x8_d = nc.dram_tensor("x8", [BS, C_IN, 2, L8], FP8,
                          kind="ExternalInput").ap()
    # [E | F] columns, fp32
    ef_d = nc.dram_tensor("ef", [BS, 2 * C_IN, 1], F32R,
                          kind="ExternalInput").ap()
    MS_d = nc.dram_tensor("MS", [C_IN, 320], F32R, kind="ExternalInput").ap()
    MEF_d = nc.dram_tensor("MEF", [2 * C_IN, 320], F32R,
                           kind="ExternalInput").ap()
    MbS_d = nc.dram_tensor("MbS", [C_IN, C_OUT], F32,
                           kind="ExternalInput").ap()
    MbEF_d = nc.dram_tensor("MbEF", [2 * C_IN, C_OUT], F32,
                            kind="ExternalInput").ap()
    y_d = nc.dram_tensor("y", [BS, C_OUT, L], BF16,
                         kind="ExternalOutput").ap()

    with tile.TileContext(nc) as tc:
        with (
            tc.tile_pool(name="consts", bufs=1) as consts,
            tc.tile_pool(name="xp", bufs=2) as xp,
            tc.tile_pool(name="x8p", bufs=2) as x8p,
            tc.tile_pool(name="yp", bufs=2) as yp,
            tc.tile_pool(name="small", bufs=2) as small,
            tc.tile_pool(name="ps", bufs=4, space="PSUM") as psy,
        ):
            M_S = consts.tile([C_IN, 320], F32R)
            M_EF = consts.tile([2 * C_IN, 320], F32R)
            Mb_S = consts.tile([C_IN, C_OUT], F32)
            Mb_EF = consts.tile([2 * C_IN, C_OUT], F32)
            dump = consts.tile([C_IN, CW + 2], BF16)  # ACT-reduce dummy out
            nc.sync.dma_start(M_S[:], MS_d)
            nc.sync.dma_start(M_EF[:], MEF_d)
            nc.sync.dma_start(Mb_S[:], MbS_d)
            nc.sync.dma_start(Mb_EF[:], MbEF_d)

            for b in range(BS):
                # ---- loads: bf16 stats copy (4 chunks), fp8 conv copy
                # ---- (2 chunks), E/F columns ----
                xs = xp.tile([C_IN, L + 2], BF16, tag="xs")
                x8 = x8p.tile([C_IN, 2, L8], FP8, tag="x8")
                colB = small.tile([2 * C_IN, 1], F32R, tag="colB")
                colP = small.tile([C_IN, NCHUNK], F32, tag="colP")
                nc.sync.dma_start(colB[:], ef_d[b])
                for c in range(NCHUNK):
                    c0 = c * CW
                    c1 = (c + 1) * CW if c < NCHUNK - 1 else L + 2
                    nc.sync.dma_start(xs[:, c0:c1], x_d[b][:, c0:c1])
                    if c % 2 == 0:  # DVE takes chunks 0, 2
                        nc.vector.reduce_sum(out=colP[:, c:c + 1],
                                             in_=xs[:, c0:c1],
                                             axis=mybir.AxisListType.X)
                    else:           # ACT takes chunks 1, 3 via accumulator
                        nc.scalar.activation(
                            dump[:, 0:c1 - c0], xs[:, c0:c1],
                            mybir.ActivationFunctionType.Identity,
                            accum_out=colP[:, c:c + 1])
                for c in range(2):
                    h0 = c * L8 // 2
                    h1 = (c + 1) * L8 // 2
                    nc.gpsimd.dma_start(x8[:, :, h0:h1],
                                        x8_d[b][:, :, h0:h1])

                colS = small.tile([C_IN, 1], F32R, tag="colS")
                with nc.allow_low_precision(reason="4-elem fp32 sum"):
                    nc.vector.reduce_sum(out=colS[:], in_=colP[:],
                                         axis=mybir.AxisListType.X)

                # ---- synthesis in sub-ranges of one 4-bank PSUM tile ----
                sy = psy.tile([C_OUT, GW], F32, tag="py")
                psp = sy[0:1, 0:320]
                psb = sy[0:C_OUT, 512:513]
                psW0 = sy[0:C_IN, 640:768]
                psW1 = sy[0:C_IN, 768:896]
                psW2 = sy[0:C_IN, 896:1024]
                nc.tensor.matmul(psp, colS[:], M_S[:],
                                 start=True, stop=False)
                nc.tensor.matmul(psp, colB[:], M_EF[:],
                                 start=False, stop=True)
                # moving dim 1 is fp32r-ISA-invalid; these two are tiny
                nc.tensor.matmul(psb, Mb_S[:], colS[:].bitcast(F32),
                                 start=True, stop=False)
                nc.tensor.matmul(psb, Mb_EF[:], colB[:].bitcast(F32),
                                 start=False, stop=True)
                params = small.tile([1, 320], BF16, tag="params")
                biasv = small.tile([C_OUT, 1], F32, tag="biasv")
                nc.scalar.activation(params[:], psp,
                                     mybir.ActivationFunctionType.Identity)
                nc.vector.tensor_copy(biasv[:], psb)

                # ---- rank-1 conv weights, scaled into fp8 range ----
                w_out_row = params[0:1, 192:320]
                nc.tensor.matmul(psW0, params[0:1, 0:64], w_out_row,
                                 start=True, stop=True)
                nc.tensor.matmul(psW1, params[0:1, 64:128], w_out_row,
                                 start=True, stop=True)
                nc.tensor.matmul(psW2, params[0:1, 128:192], w_out_row,
                                 start=True, stop=True)
                W01dr = small.tile([C_IN, 2, C_OUT], FP8, tag="W01dr")
                W2Z = small.tile([C_IN, 2, C_OUT], FP8, tag="W2Z")
                nc.scalar.activation(W01dr[:, 0, :], psW0,
                                     mybir.ActivationFunctionType.Identity,
                                     scale=WSCALE)
                nc.scalar.activation(W01dr[:, 1, :], psW1,
                                     mybir.ActivationFunctionType.Identity,
                                     scale=WSCALE)
                nc.vector.tensor_scalar(out=W2Z[:, 0, :], in0=psW2,
                                        scalar1=WSCALE, scalar2=None,
                                        op0=mybir.AluOpType.mult)
                nc.gpsimd.memset(W2Z[:, 1, :], 0)

                # ---- conv: 2 fp8 DoubleRow matmuls per 512 tile ----
                yb = yp.tile([C_OUT, L], BF16, tag="yb")
                for g in range(NTILES // GROUP):
                    py = psy.tile([C_OUT, GW], F32, tag="py")
                    for j in range(GROUP):
                        t = g * GROUP + j
                        nc.tensor.matmul(
                            py[:, NT * j:NT * (j + 1)], W01dr[:],
                            x8[:, :, NT * t:NT * t + NT],
                            perf_mode=DR, start=True, stop=False)
                    for j in range(GROUP):
                        t = g * GROUP + j
                        nc.tensor.matmul(
                            py[:, NT * j:NT * (j + 1)], W2Z[:],
                            x8[:, :, NT * t + 2:NT * t + 2 + NT],
                            perf_mode=DR, start=False, stop=True)
                    dst = yb[:, GW * g:GW * (g + 1)]
                    if g % 2 == 0:
                        nc.scalar.activation(
                            dst, py[:],
                            mybir.ActivationFunctionType.Identity,
                            bias=biasv[:], scale=1.0 / WSCALE)
                    else:
                        nc.vector.tensor_scalar(
                            out=dst, in0=py[:],
                            scalar1=1.0 / WSCALE, scalar2=biasv[:],
                            op0=mybir.AluOpType.mult,
                            op1=mybir.AluOpType.add)
                    nc.sync.dma_start(y_d[b][:, GW * g:GW * (g + 1)], dst)

    nc.compile()
    _CACHE["nc"] = nc
    return nc


def kernel(x, W_kernel, W_in, W_out, W_bias):
    x = np.asarray(x, dtype=np.float32)
    # one zero column each side: the device reads x[l-1], x[l], x[l+1]
    xpad = np.pad(x, [(0, 0), (0, 0), (1, 1)])
    xs = xpad.astype(BF16_NP)
    p0 = np.zeros((B, C_IN, L + 16), FP8_NP)
    p0[:, :, :L + 2] = xpad.astype(FP8_NP)
    p1 = np.concatenate([p0[:, :, 1:], np.zeros((B, C_IN, 1), FP8_NP)],
                        axis=2)
    x8 = np.stack([p0, p1], axis=2)                       # [B, 64, 2, L8]
    ef = np.concatenate([x[:, :, L - 1], x[:, :, 0]],
                        axis=1)[:, :, None].astype(np.float32)
    M_S, M_EF, Mb_S, Mb_EF = _host_precompute(
        np.asarray(W_kernel, np.float32), np.asarray(W_in, np.float32),
        np.asarray(W_out, np.float32), np.asarray(W_bias, np.float32))

    nc = _build_module()
    in_maps = [
        {"x": xs[c * BS:(c + 1) * BS], "x8": x8[c * BS:(c + 1) * BS],
         "ef": ef[c * BS:(c + 1) * BS], "MS": M_S, "MEF": M_EF,
         "MbS": Mb_S, "MbEF": Mb_EF}
        for c in range(N_CORES)
    ]
    res = run_bass_kernel_spmd(nc, in_maps, core_ids=list(range(N_CORES)))
    global LAST_RESULT
    LAST_RESULT = res
    y = np.concatenate([r["y"] for r in res.results], axis=0)
    return y.astype(np.float32)


LAST_RESULT = None


# revision 19
# speedup vs baseline: 1.1906x; 1.1906x over previous
"""Trainium2 Bass kernel for nn_ConvPlus1d (dense_cnn).

Algorithm (mathematically identical to the reference, derived analytically):

  The reference synthesizes per-sample conv weights:
      kern[b]   = mean_L(depthwise_conv(x))        -> [B, C_IN, K]
      w_in[b]   = W_in @ kern[b]                   -> [B, C_IN, K]
      w_out[b]  = <W_out, kern[b]>                 -> [B, C_OUT]
      bias[b]   = <W_bias, kern[b]>                -> [B, C_OUT]
      weight[b, o, c, k] = w_in[b, c, k] * w_out[b, o]     (rank-1!)
      y[b] = conv1d(x[b], weight[b], pad=1) + bias[b]

  Exact simplifications used here:
  1) mean over L of a pad-1 depthwise conv only needs per-channel sums and
     the first/last elements: kern (and therefore all synthesized params)
     are LINEAR in (S, E, F) with host-precomputable coefficient matrices.
     E/F are single input columns, shipped pre-gathered; S is reduced on
     device from the bf16 copy of x.
  2) The per-sample conv weight is rank-1 across (o) x (c,k).

  Device program per sample (data-parallel over batch, 4 samples/core):
      xs bf16 [64, L+2]  (stats), x8 fp8 [64, 2, L+2] (conv; plane 1 is
      plane 0 shifted left one column, so a DoubleRow matmul consumes two
      taps at 0.5 cycles/row)
      S: 4 chunk reduces (2 on DVE, 2 via ACT accum_out)
      params[1,320] = S^T M_S + [E|F]^T M_EF      (PE fp32r)
      bias[128,1]   = Mb_S^T S + Mb_EF^T [E|F]    (PE fp32, column out)
      W01dr fp8 [64, 2*128] = [w_in_k0 | w_in_k1] outer w_out, x 2^16
      W2Z   fp8 [64, 2*128] = [w_in_k2 | 0] outer w_out, x 2^16
      conv: per 512 tile, 2 fp8 DoubleRow matmuls (taps 0+1, tap 2+zero)
      PSUM is a 4-bank [128,2048] tile per 4-tile group; one eviction
      instruction per group applies x 2^-16 and the bias (ACT/DVE
      alternating), writing y in bf16.
  Host upcasts y bf16 -> fp32 (rel tol is 2e-2; measured pipeline error
  ~2e-3, dominated by bf16 quantization of x and the bf16 y store).

Sharding: batch 32 -> 8 cores x 4 samples, maker params replicated.
"""

import sys

import ml_dtypes
import numpy as np

sys.path.insert(0, "/opt/trn_rl_repo")

import concourse.bacc as bacc  # noqa: E402
import concourse.tile as tile  # noqa: E402
from concourse import mybir  # noqa: E402
from concourse.bass_utils import run_bass_kernel_spmd  # noqa: E402

B, C_IN, C_OUT, K, L = 32, 64, 128, 3, 8192
N_CORES = 8
BS = B // N_CORES          # samples per core
NT = 512                   # matmul moving-dim tile (one PSUM bank of fp32)
NTILES = L // NT
GROUP = 4                  # conv tiles per 4-bank PSUM tile
GW = NT * GROUP            # 2048 output columns per group
NCHUNK = 4                 # x load / reduce chunks
CW = 2048                  # chunk width (last chunk is CW+2)
WSCALE = 65536.0           # fp8 weight scale (Wtap rms ~3e-7 -> ~0.02)
L8 = L + 16                # fp8 plane length, 16B-aligned (pad cols are 0)

F32 = mybir.dt.float32
F32R = mybir.dt.float32r
BF16 = mybir.dt.bfloat16
FP8 = mybir.dt.float8e4
BF16_NP = ml_dtypes.bfloat16
FP8_NP = ml_dtypes.float8_e4m3
DR = mybir.MatmulPerfMode.DoubleRow


def _host_precompute(W_kernel, W_in, W_out, W_bias):
    """Fold the maker parameters into linear maps on the stats (S, E, F).

    params layout: [w_in k=0 (64) | k=1 (64) | k=2 (64) | w_out (128)].
    Returns M_S [64,320], M_EF [128,320] (rows 0:64 E, 64:128 F coeffs),
    Mb_S [64,128], Mb_EF [128,128].
    """
    Wk = W_kernel.reshape(C_IN, K, K).astype(np.float64)     # [c, j, t]
    P = (Wk[:, :, 0] + Wk[:, :, 1] + Wk[:, :, 2]) / L        # coeff on S
    Q = -Wk[:, :, 0] / L                                     # coeff on E
    R = -Wk[:, :, 2] / L                                     # coeff on F

    Win = W_in[:, :, 0].astype(np.float64)                   # [c, c']

    def m_in(Xc):   # -> [c', k*64+c]
        return np.einsum("cp,pk->pkc", Win, Xc).reshape(C_IN, K * C_IN)

    def m_out(Xc, W):  # -> [c', o]
        return np.einsum("ock,ck->co", W.astype(np.float64), Xc)

    def mm(Xc):
        return np.concatenate([m_in(Xc), m_out(Xc, W_out)], axis=1)  # [64,320]

    M_S = mm(P).astype(np.float32)
    M_EF = np.concatenate([mm(Q), mm(R)], axis=0).astype(np.float32)
    Mb_S = m_out(P, W_bias).astype(np.float32)
    Mb_EF = np.concatenate(
        [m_out(Q, W_bias), m_out(R, W_bias)], axis=0).astype(np.float32)
    return M_S, M_EF, Mb_S, Mb_EF


_CACHE = {}


def _build_module():
    if "nc" in _CACHE:
        return _CACHE["nc"]
    nc = bacc.Bacc("TRN2", target_bir_lowering=False, debug=False)

    # host supplies x pre-padded with one zero column on each side
    x_d = nc.dram_tensor("x", [BS, C_IN, L + 2], BF16,
                         kind="ExternalInput").ap()
    # fp8 conv input: plane 0 = xpad, plane 1 = xpad shifted left by 1
    x8_d = nc.dram_tensor("x8", [BS, C_IN, 2, L8], FP8,
                          kind="ExternalInput").ap()
    # [E | F] columns, fp32
    ef_d = nc.dram_tensor("ef", [BS, 2 * C_IN, 1], F32R,
                          kind="ExternalInput").ap()
    MS_d = nc.dram_tensor("MS", [C_IN, 320], F32R, kind="ExternalInput").ap()
    MEF_d = nc.dram_tensor("MEF", [2 * C_IN, 320], F32R,
                           kind="ExternalInput").ap()
    MbS_d = nc.dram_tensor("MbS", [C_IN, C_OUT], F32,
                           kind="ExternalInput").ap()
    MbEF_d = nc.dram_tensor("MbEF", [2 * C_IN, C_OUT], F32,
                            kind="ExternalInput").ap()
    y_d = nc.dram_tensor("y", [BS, C_OUT, L], BF16,
                         kind="ExternalOutput").ap()

    with tile.TileContext(nc) as tc:
        with (
            tc.tile_pool(name="consts", bufs=1) as consts,
            tc.tile_pool(name="xp", bufs=2) as xp,
            tc.tile_pool(name="x8p", bufs=2) as x8p,
            tc.tile_pool(name="yp", bufs=2) as yp,
            tc.tile_pool(name="small", bufs=2) as small,
            tc.tile_pool(name="ps", bufs=2, space="PSUM") as psy,
        ):
            M_S = consts.tile([C_IN, 320], F32R)
            M_EF = consts.tile([2 * C_IN, 320], F32R)
            Mb_S = consts.tile([C_IN, C_OUT], F32)
            Mb_EF = consts.tile([2 * C_IN, C_OUT], F32)
            dump = consts.tile([C_IN, CW + 2], BF16)  # ACT-reduce dummy out
            nc.sync.dma_start(M_S[:], MS_d)
            nc.sync.dma_start(M_EF[:], MEF_d)
            nc.sync.dma_start(Mb_S[:], MbS_d)
            nc.sync.dma_start(Mb_EF[:], MbEF_d)

            for b in range(BS):
                # ---- loads: bf16 stats copy (4 chunks), fp8 conv copy
                # ---- (2 chunks), E/F columns ----
                xs = xp.tile([C_IN, L + 2], BF16, tag="xs")
                x8 = x8p.tile([C_IN, 2, L8], FP8, tag="x8")
                colB = small.tile([2 * C_IN, 1], F32R, tag="colB")
                colP = small.tile([C_IN, NCHUNK], F32, tag="colP")
                nc.sync.dma_start(colB[:], ef_d[b])
                for c in range(NCHUNK):
                    c0 = c * CW
                    c1 = (c + 1) * CW if c < NCHUNK - 1 else L + 2
                    nc.sync.dma_start(xs[:, c0:c1], x_d[b][:, c0:c1])
                    if c % 2 == 0:  # DVE takes chunks 0, 2
                        nc.vector.reduce_sum(out=colP[:, c:c + 1],
                                             in_=xs[:, c0:c1],
                                             axis=mybir.AxisListType.X)
                    else:           # ACT takes chunks 1, 3 via accumulator
                        nc.scalar.activation(
                            dump[:, 0:c1 - c0], xs[:, c0:c1],
                            mybir.ActivationFunctionType.Identity,
                            accum_out=colP[:, c:c + 1])
                for c in range(2):
                    h0 = c * L8 // 2
                    h1 = (c + 1) * L8 // 2
                    nc.gpsimd.dma_start(x8[:, :, h0:h1],
                                        x8_d[b][:, :, h0:h1])

                colS = small.tile([C_IN, 1], F32R, tag="colS")
                with nc.allow_low_precision(reason="4-elem fp32 sum"):
                    nc.vector.reduce_sum(out=colS[:], in_=colP[:],
                                         axis=mybir.AxisListType.X)

                # ---- synthesis in sub-ranges of one 4-bank PSUM tile ----
                sy = psy.tile([C_OUT, GW], F32, tag="py")
                psp = sy[0:1, 0:320]
                psb = sy[0:C_OUT, 512:513]
                psW0 = sy[0:C_IN, 1024:1152]
                psW1 = sy[0:C_IN, 1152:1280]
                psW2 = sy[0:C_IN, 1536:1664]
                nc.tensor.matmul(psp, colS[:], M_S[:],
                                 start=True, stop=False)
                nc.tensor.matmul(psp, colB[:], M_EF[:],
                                 start=False, stop=True)
                # moving dim 1 is fp32r-ISA-invalid; these two are tiny
                nc.tensor.matmul(psb, Mb_S[:], colS[:].bitcast(F32),
                                 start=True, stop=False)
                nc.tensor.matmul(psb, Mb_EF[:], colB[:].bitcast(F32),
                                 start=False, stop=True)
                params = small.tile([1, 320], BF16, tag="params")
                biasv = small.tile([C_OUT, 1], F32, tag="biasv")
                nc.scalar.activation(params[:], psp,
                                     mybir.ActivationFunctionType.Identity)
                nc.vector.tensor_copy(biasv[:], psb)

                # ---- rank-1 conv weights, scaled into fp8 range ----
                w_out_row = params[0:1, 192:320]
                nc.tensor.matmul(psW0, params[0:1, 0:64], w_out_row,
                                 start=True, stop=True)
                nc.tensor.matmul(psW1, params[0:1, 64:128], w_out_row,
                                 start=True, stop=True)
                nc.tensor.matmul(psW2, params[0:1, 128:192], w_out_row,
                                 start=True, stop=True)
                W01dr = small.tile([C_IN, 2, C_OUT], FP8, tag="W01dr")
                W2Z = small.tile([C_IN, 2, C_OUT], FP8, tag="W2Z")
                nc.scalar.activation(W01dr[:, 0, :], psW0,
                                     mybir.ActivationFunctionType.Identity,
                                     scale=WSCALE)
                nc.scalar.activation(W01dr[:, 1, :], psW1,
                                     mybir.ActivationFunctionType.Identity,
                                     scale=WSCALE)
                nc.vector.tensor_scalar(out=W2Z[:, 0, :], in0=psW2,
                                        scalar1=WSCALE, scalar2=None,
                                        op0=mybir.AluOpType.mult)
                nc.gpsimd.memset(W2Z[:, 1, :], 0)

                # ---- conv: 2 fp8 DoubleRow matmuls per 512 tile ----
                yb = yp.tile([C_OUT, L], BF16, tag="yb")
                for g in range(NTILES // GROUP):
                    py = psy.tile([C_OUT, GW], F32, tag="py")
                    for j in range(GROUP):
                        t = g * GROUP + j
                        nc.tensor.matmul(
                            py[:, NT * j:NT * (j + 1)], W01dr[:],
                            x8[:, :, NT * t:NT * t + NT],
                            perf_mode=DR, start=True, stop=False)
                    for j in range(GROUP):
                        t = g * GROUP + j
                        nc.tensor.matmul(
                            py[:, NT * j:NT * (j + 1)], W2Z[:],
                            x8[:, :, NT * t + 2:NT * t + 2 + NT],
                            perf_mode=DR, start=False, stop=True)
                    dst = yb[:, GW * g:GW * (g + 1)]
                    if g % 2 == 0:
                        nc.scalar.activation(
                            dst, py[:],
                            mybir.ActivationFunctionType.Identity,
                            bias=biasv[:], scale=1.0 / WSCALE)
                    else:
                        nc.vector.tensor_scalar(
                            out=dst, in0=py[:],
                            scalar1=1.0 / WSCALE, scalar2=biasv[:],
                            op0=mybir.AluOpType.mult,
                            op1=mybir.AluOpType.add)
                    nc.sync.dma_start(y_d[b][:, GW * g:GW * (g + 1)], dst)

    nc.compile()
    _CACHE["nc"] = nc
    return nc


def kernel(x, W_kernel, W_in, W_out, W_bias):
    x = np.asarray(x, dtype=np.float32)
    # one zero column each side: the device reads x[l-1], x[l], x[l+1]
    xpad = np.pad(x, [(0, 0), (0, 0), (1, 1)])
    xs = xpad.astype(BF16_NP)
    p0 = np.zeros((B, C_IN, L + 16), FP8_NP)
    p0[:, :, :L + 2] = xpad.astype(FP8_NP)
    p1 = np.concatenate([p0[:, :, 1:], np.zeros((B, C_IN, 1), FP8_NP)],
                        axis=2)
    x8 = np.stack([p0, p1], axis=2)                       # [B, 64, 2, L8]
    ef = np.concatenate([x[:, :, L - 1], x[:, :, 0]],
                        axis=1)[:, :, None].astype(np.float32)
    M_S, M_EF, Mb_S, Mb_EF = _host_precompute(
        np.asarray(W_kernel, np.float32), np.asarray(W_in, np.float32),
        np.asarray(W_out, np.float32), np.asarray(W_bias, np.float32))

    nc = _build_module()
    in_maps = [
        {"x": xs[c * BS:(c + 1) * BS], "x8": x8[c * BS:(c + 1) * BS],
         "ef": ef[c * BS:(c + 1) * BS], "MS": M_S, "MEF": M_EF,
         "MbS": Mb_S, "MbEF": Mb_EF}
        for c in range(N_CORES)
    ]
    res = run_bass_kernel_spmd(nc, in_maps, core_ids=list(range(N_CORES)))
    global LAST_RESULT
    LAST_RESULT = res
    y = np.concatenate([r["y"] for r in res.results], axis=0)
    return y.astype(np.float32)


LAST_RESULT = None
